# revision 1
# baseline (speedup 1.0000x reference)
"""3-layer GCN node predictor on 8 Trainium2 NeuronCores (Bass/Tile SPMD).

Strategy (graph/data parallel, per sharding hint):
- Nodes sharded into 8 contiguous chunks (12544 padded rows per core); each
  core aggregates the in-edges of its own dst nodes.
- Per layer, the gather table T_L = o_{L-1} @ W_L ([100352, 64] fp32, 256B
  rows) is built shard-wise and AllGathered to every core's DRAM.
- Per-edge gather of T_L[src] uses gpsimd dma_gather (int16 indices ->
  4 table quarters of 25088 rows), round-robin over 4 SWDGE queues.
- Scatter-add uses TensorE: one-hot S [128 edges, 64 dst] built on DVE via
  is_equal against an iota row, matmul S.T @ msg accumulated in PSUM.
- Self loops are applied node-wise from the SBUF-resident own chunk.
"""
import numpy as np

import concourse.bass as bass
import concourse.bacc as bacc
import concourse.tile as tile
import concourse.mybir as mybir
from concourse.bass_utils import run_bass_kernel_spmd

NCORES = 8
N = 100000
E = 3200000
F_IN = 128
HID = 32
NCLS = 10
RC = 12500          # real nodes per core
PC = 12544          # padded nodes per core (98 * 128)
NP = PC * NCORES    # padded total nodes (100352)
Q4 = NP // 4        # table quarter rows (25088), int16-addressable
ELEM = 64           # table row elements (256B rows)
W = 64              # dst window
NWIN = PC // W      # 196 windows per core
SGW = 4             # windows per supergroup
NSG = NWIN // SGW   # 49
NG = PC // 128      # 98 node groups of 128
EPS2 = 1e-24

_cache = {}


def _host_prep(x, edge_index, edge_weights):
    src = np.asarray(edge_index[0], dtype=np.int64)
    dst = np.asarray(edge_index[1], dtype=np.int64)
    ew = np.asarray(edge_weights, dtype=np.float64)

    deg = np.bincount(dst, weights=ew, minlength=N) + 1.0
    dinv = np.where(deg > 0, 1.0 / np.sqrt(deg), 0.0)
    cnorm_e = (dinv[src] * ew * dinv[dst]).astype(np.float32)
    s2 = (dinv * dinv).astype(np.float32)

    psrc = (src // RC) * PC + (src % RC)          # padded global src ids

    per_core = []
    for c in range(NCORES):
        m = (dst >= RC * c) & (dst < RC * (c + 1))
        es = psrc[m]
        ed = dst[m] - RC * c
        en = cnorm_e[m]
        w_id = ed // W
        q_id = es // Q4
        order = np.lexsort((ed, q_id, w_id))      # sort by (w, q, dst)
        per_core.append((es[order], ed[order], en[order],
                         w_id[order], q_id[order]))

    # per (w, q) counts and max over cores
    counts = np.zeros((NCORES, NWIN, 4), dtype=np.int64)
    for c in range(NCORES):
        _, _, _, w_id, q_id = per_core[c]
        np.add.at(counts[c], (w_id, q_id), 1)
    cmax = counts.max(axis=0)
    t_wq = (cmax + 127) // 128                    # tiles per (w, q)
    for w_i in range(NWIN):
        if t_wq[w_i].sum() == 0:
            t_wq[w_i, 0] = 1

    # global tile order: (sg, q, w, k)
    tile_of = {}
    T_total = 0
    call_meta = []                                # (sg, q, t0, ntiles)
    for sg in range(NSG):
        for q in range(4):
            t0 = T_total
            for w_i in range(sg * SGW, (sg + 1) * SGW):
                for k in range(t_wq[w_i, q]):
                    tile_of[(w_i, q, k)] = T_total
                    T_total += 1
            call_meta.append((sg, q, t0, T_total - t0))

    # slot arrays
    idx16 = np.zeros((T_total * 128,), dtype=np.int16)
    cnorm = np.zeros((T_total * 128,), dtype=np.float32)
    dstrel = np.full((T_total * 128,), -1.0, dtype=np.float32)
    idx16_all = np.zeros((NCORES, T_total * 128), dtype=np.int16)
    cnorm_all = np.zeros((NCORES, T_total * 128), dtype=np.float32)
    dstrel_all = np.full((NCORES, T_total * 128), -1.0, dtype=np.float32)
    for c in range(NCORES):
        es, ed, en, w_id, q_id = per_core[c]
        # position within (w, q) run
        keys = w_id * 4 + q_id
        # edges already sorted by (w, q); rank within group:
        boundaries = np.flatnonzero(np.diff(keys, prepend=-1))
        ranks = np.arange(len(keys)) - np.repeat(boundaries, np.diff(np.append(boundaries, len(keys))))
        k_tile = ranks // 128
        k_part = ranks % 128
        gtile = np.array([tile_of[(w, q, k)] for (w, q, k) in zip(w_id, q_id, k_tile)])
        slot = gtile * 128 + k_part
        idx16_all[c, slot] = (es % Q4).astype(np.int16)
        cnorm_all[c, slot] = en
        dstrel_all[c, slot] = (ed - w_id * W).astype(np.float32)

    # device layouts
    # cnorm/dstrel resident [128, T]: flat p * T + t; slot = t*128 + p
    def to_pt(a):
        return np.ascontiguousarray(a.reshape(-1, T_total, 128).transpose(0, 2, 1)).reshape(NCORES, -1)

    cnorm_pt = to_pt(cnorm_all)
    dstrel_pt = to_pt(dstrel_all)

    # idx per call: wrapped [128, 8*ntiles] int16, idx j of call at [j%16, j//16],
    # replicated across the 8 groups of 16 partitions. Flattened per call.
    idx_blocks = np.zeros((NCORES, T_total * 1024), dtype=np.int16)
    for (sg, q, t0, nt) in call_meta:
        if nt == 0:
            continue
        nidx = nt * 128
        for c in range(NCORES):
            blk = idx16_all[c, t0 * 128:(t0 + nt) * 128]
            wrp = blk.reshape(nidx // 16, 16).T              # [16, nidx/16]
            rep = np.tile(wrp, (8, 1))                       # [128, nidx/16]
            idx_blocks[c, t0 * 1024:t0 * 1024 + nidx * 8] = rep.reshape(-1)

    # s2/dinv resident [128, NG]: flat p * NG + g ; node g*128+p
    def node_pt(v):
        pad = np.zeros((NCORES, PC), dtype=np.float32)
        for c in range(NCORES):
            pad[c, :RC] = v[RC * c:RC * (c + 1)]
        return np.ascontiguousarray(pad.reshape(NCORES, NG, 128).transpose(0, 2, 1)).reshape(NCORES, -1)

    s2_pt = node_pt(s2)
    dinv_pt = node_pt(dinv.astype(np.float32))
    ones_fast = bool(np.all(np.asarray(edge_weights) == 1.0))

    # x chunks
    x = np.asarray(x, dtype=np.float32)
    x_pad = np.zeros((NCORES, PC, F_IN), dtype=np.float32)
    for c in range(NCORES):
        x_pad[c, :RC] = x[RC * c:RC * (c + 1)]

    return dict(
        T_total=T_total, t_wq=t_wq, tile_of=tile_of, call_meta=call_meta,
        cnorm_pt=cnorm_pt, dstrel_pt=dstrel_pt, idx_blocks=idx_blocks,
        s2_pt=s2_pt, dinv_pt=dinv_pt, ones_fast=ones_fast, x_pad=x_pad,
    )


def _build_program(meta, reps=1, skip_gather=False, skip_scatter=False, ones_fast=False, debug_o1=False, dbgL=0):
    T_total = meta["T_total"]
    t_wq = meta["t_wq"]
    tile_of = meta["tile_of"]
    call_meta = meta["call_meta"]
    f32 = mybir.dt.float32

    nc = bacc.Bacc("TRN2", target_bir_lowering=False, debug=False,
                   num_devices=NCORES, num_swdge_queues=4)

    t_x = nc.dram_tensor("x_c", [PC * F_IN], f32, kind="ExternalInput").ap()
    t_idx = nc.dram_tensor("idxb", [T_total * 1024], mybir.dt.int16, kind="ExternalInput").ap()
    t_cnorm = nc.dram_tensor("cnorm", [128 * T_total], f32, kind="ExternalInput").ap()
    t_dstrel = nc.dram_tensor("dstrel", [128 * T_total], f32, kind="ExternalInput").ap()
    t_s2 = nc.dram_tensor("s2", [128 * NG], f32, kind="ExternalInput").ap()
    t_dinv = nc.dram_tensor("dinv", [128 * NG], f32, kind="ExternalInput").ap()
    t_w1 = nc.dram_tensor("w1", [F_IN, HID], f32, kind="ExternalInput").ap()
    t_w2 = nc.dram_tensor("w2", [HID, HID], f32, kind="ExternalInput").ap()
    t_w3 = nc.dram_tensor("w3", [HID, HID], f32, kind="ExternalInput").ap()
    t_wl = nc.dram_tensor("wl", [3 * HID, NCLS], f32, kind="ExternalInput").ap()
    t_brep = nc.dram_tensor("brep", [3, 128, HID], f32, kind="ExternalInput").ap()
    t_blrep = nc.dram_tensor("blrep", [128, NCLS], f32, kind="ExternalInput").ap()
    t_iota = nc.dram_tensor("iota", [128, W], f32, kind="ExternalInput").ap()
    t_eye = nc.dram_tensor("eye", [128, 128], f32, kind="ExternalInput").ap()
    t_y = nc.dram_tensor("y", [PC * NCLS], f32, kind="ExternalOutput").ap()

    with tile.TileContext(nc) as tc:
        with (
            tc.tile_pool(name="const", bufs=1) as cp,
            tc.tile_pool(name="resident", bufs=1) as rp,
            tc.tile_pool(name="work", bufs=3) as wp,
            tc.tile_pool(name="msgp", bufs=2) as mp,
            tc.tile_pool(name="sp", bufs=6) as spool,
            tc.tile_pool(name="psum", bufs=2, space="PSUM") as pp,
            tc.tile_pool(name="psum2", bufs=2, space="PSUM") as pp2,
            tc.tile_pool(name="dram", bufs=1, space="DRAM") as dp,
        ):
            # ---- constants / residents ----
            w1_t = cp.tile([F_IN, HID], f32); nc.sync.dma_start(out=w1_t[:], in_=t_w1[:])
            w2_t = cp.tile([HID, HID], f32); nc.sync.dma_start(out=w2_t[:], in_=t_w2[:])
            w3_t = cp.tile([HID, HID], f32); nc.sync.dma_start(out=w3_t[:], in_=t_w3[:])
            wl_ts = []
            for L in range(3):
                wt = cp.tile([HID, NCLS], f32, tag=f"wl{L}", name=f"wl{L}")
                nc.sync.dma_start(out=wt[:], in_=t_wl[HID * L:HID * (L + 1), :])
                wl_ts.append(wt)
            brep_t = cp.tile([128, 3, HID], f32)
            nc.sync.dma_start(out=brep_t[:], in_=t_brep.rearrange("l p h -> p l h"))
            blrep_t = cp.tile([128, NCLS], f32); nc.sync.dma_start(out=blrep_t[:], in_=t_blrep[:])
            iota_t = cp.tile([128, W], f32); nc.sync.dma_start(out=iota_t[:], in_=t_iota[:])
            eye_t = cp.tile([128, 128], f32); nc.sync.dma_start(out=eye_t[:], in_=t_eye[:])
            cnorm_t = rp.tile([128, T_total], f32)
            nc.sync.dma_start(out=cnorm_t[:], in_=t_cnorm.rearrange("(p t) -> p t", t=T_total))
            dstrel_t = rp.tile([128, T_total], f32)
            nc.sync.dma_start(out=dstrel_t[:], in_=t_dstrel.rearrange("(p t) -> p t", t=T_total))
            s2_t = rp.tile([128, NG], f32)
            nc.sync.dma_start(out=s2_t[:], in_=t_s2.rearrange("(p g) -> p g", g=NG))
            dinv_t = rp.tile([128, NG], f32)
            nc.sync.dma_start(out=dinv_t[:], in_=t_dinv.rearrange("(p g) -> p g", g=NG))

            h_own = rp.tile([128, NG, HID], f32)          # own chunk of current table (pre-pad)
            o_bufs = [rp.tile([128, NG, HID], f32, tag=f"o{L}", name=f"o{L}") for L in range(3)]

            tables = [dp.tile([NP * ELEM], f32, tag=f"table{L}", name=f"table{L}") for L in range(3)]
            in_bs = [dp.tile([PC * ELEM], f32, tag=f"inb{L}", name=f"inb{L}") for L in range(3)]

            for _rep in range(reps):
                x_v = t_x.rearrange("(g p f) -> g p f", p=128, f=F_IN)

                def chain_write(L, g, o_ap):
                    """From o tile [128, HID] compute h = o @ W_{L+1}, write padded
                    row block to in_bs[L+1] and h_own."""
                    wn = [None, w2_t, w3_t][L + 1]
                    ot_ps = pp2.tile([HID, 128], f32, tag="tps")
                    nc.tensor.transpose(out=ot_ps[:], in_=o_ap, identity=eye_t[:])
                    ot_sb = wp.tile([HID, 128], f32, tag="otsb")
                    nc.vector.tensor_copy(out=ot_sb[:], in_=ot_ps[:])
                    h_ps = pp.tile([128, HID], f32, tag="hps")
                    nc.tensor.matmul(h_ps[:], lhsT=ot_sb[:], rhs=wn[:], start=True, stop=True)
                    h64 = wp.tile([128, ELEM], f32, tag="h64")
                    nc.vector.memset(h64[:, HID:], 0.0)
                    if ones_fast:
                        nc.vector.tensor_scalar(out=h64[:, :HID], in0=h_ps[:],
                                                scalar1=dinv_t[:, g:g + 1], scalar2=None,
                                                op0=mybir.AluOpType.mult)
                    else:
                        nc.vector.tensor_copy(out=h64[:, :HID], in_=h_ps[:])
                    nc.vector.tensor_copy(out=h_own[:, g, :], in_=h64[:, :HID])
                    nc.sync.dma_start(
                        out=in_bs[L + 1][:].rearrange("(g p e) -> g p e", p=128, e=ELEM)[g],
                        in_=h64[:])

                # ---- layer 1 table: h1 = x @ W1 ----
                for g in range(NG):
                    xt = wp.tile([128, F_IN], f32, tag="xt")
                    nc.sync.dma_start(out=xt[:], in_=x_v[g])
                    xT_ps = pp2.tile([128, 128], f32, tag="tps")
                    nc.tensor.transpose(out=xT_ps[:], in_=xt[:], identity=eye_t[:])
                    xT_sb = wp.tile([128, F_IN], f32, tag="xTsb")
                    nc.vector.tensor_copy(out=xT_sb[:], in_=xT_ps[:])
                    h_ps = pp.tile([128, HID], f32, tag="hps")
                    nc.tensor.matmul(h_ps[:], lhsT=xT_sb[:], rhs=w1_t[:], start=True, stop=True)
                    h64 = wp.tile([128, ELEM], f32, tag="h64")
                    nc.vector.memset(h64[:, HID:], 0.0)
                    if ones_fast:
                        nc.vector.tensor_scalar(out=h64[:, :HID], in0=h_ps[:],
                                                scalar1=dinv_t[:, g:g + 1], scalar2=None,
                                                op0=mybir.AluOpType.mult)
                    else:
                        nc.vector.tensor_copy(out=h64[:, :HID], in_=h_ps[:])
                    nc.vector.tensor_copy(out=h_own[:, g, :], in_=h64[:, :HID])
                    nc.sync.dma_start(
                        out=in_bs[0][:].rearrange("(g p e) -> g p e", p=128, e=ELEM)[g],
                        in_=h64[:])

                # ---- layers ----
                for L in range(3):
                    nc.gpsimd.collective_compute(
                        "AllGather", mybir.AluOpType.bypass,
                        replica_groups=[list(range(NCORES))],
                        ins=[in_bs[L][:]], outs=[tables[L][:]])
                    tab_q = [tables[L][:].rearrange("(n e) -> n e", e=ELEM)[Q4 * q:Q4 * (q + 1)]
                             for q in range(4)]

                    # supergroup tile extents
                    sg_t0 = [min(t0 for (s, q, t0, nt) in call_meta if s == sg)
                             for sg in range(NSG)]
                    sg_t1 = [max(t0 + nt for (s, q, t0, nt) in call_meta if s == sg)
                             for sg in range(NSG)]

                    for sg in range(NSG):
                        t0s, t1s = sg_t0[sg], sg_t1[sg]
                        nts = t1s - t0s
                        msg = mp.tile([128, nts, ELEM], f32, tag="msg")
                        for (s, q, t0, nt) in call_meta:
                            if s != sg or nt == 0:
                                continue
                            nidx = nt * 128
                            idxt = wp.tile([128, nt * 8], mybir.dt.int16, tag="idxt")
                            nc.sync.dma_start(
                                out=idxt[:],
                                in_=t_idx[t0 * 1024:t0 * 1024 + nidx * 8]
                                    .rearrange("(p n) -> p n", p=128))
                            if not skip_gather:
                                nc.gpsimd.dma_gather(
                                    out_ap=msg[:, t0 - t0s:t0 - t0s + nt, :],
                                    in_ap=tab_q[q],
                                    idxs_ap=idxt[:],
                                    num_idxs=nidx, num_idxs_reg=nidx,
                                    elem_size=ELEM, elem_step=ELEM,
                                    single_packet=False, queue_num=q)
                            if not ones_fast:
                                nc.vector.tensor_tensor(
                                    out=msg[:, t0 - t0s:t0 - t0s + nt, :HID],
                                    in0=msg[:, t0 - t0s:t0 - t0s + nt, :HID],
                                    in1=cnorm_t[:, t0:t0 + nt].unsqueeze(-1).to_broadcast([128, nt, HID]),
                                    op=mybir.AluOpType.mult)

                        # two psum groups (2 windows each) per supergroup
                        for half in range(SGW // 2):
                            agg_ps = pp.tile([128, HID], f32, tag="agg")
                            if skip_scatter:
                                nc.vector.memset(agg_ps[:], 0.0)
                            g = sg * 2 + half  # node group index (128 dst)
                            for wi in range(2):
                                w_i = sg * SGW + half * 2 + wi
                                tiles_w = []
                                for q in range(4):
                                    for k in range(t_wq[w_i, q]):
                                        tiles_w.append(tile_of[(w_i, q, k)])
                                # S tiles in runs of up to 8 consecutive
                                runs = []
                                for t in tiles_w:
                                    if runs and runs[-1][0] + runs[-1][1] == t and runs[-1][1] < 8:
                                        runs[-1] = (runs[-1][0], runs[-1][1] + 1)
                                    else:
                                        runs.append((t, 1))
                                s_tiles = {}
                                for (rt, rn) in (runs if not skip_scatter else []):
                                    st = spool.tile([128, 8, W], f32, tag="S")
                                    nc.vector.tensor_tensor(
                                        out=st[:, :rn, :],
                                        in0=dstrel_t[:, rt:rt + rn].unsqueeze(-1).to_broadcast([128, rn, W]),
                                        in1=iota_t[:].unsqueeze(1).to_broadcast([128, rn, W]),
                                        op=mybir.AluOpType.is_equal)
                                    for j in range(rn):
                                        s_tiles[rt + j] = st[:, j, :]
                                for ti, t in (enumerate(tiles_w) if not skip_scatter else []):
                                    nc.tensor.matmul(
                                        agg_ps[64 * wi:64 * (wi + 1), :],
                                        lhsT=s_tiles[t],
                                        rhs=msg[:, t - t0s, :HID],
                                        start=(ti == 0), stop=(ti == len(tiles_w) - 1),
                                        skip_group_check=True)
                            # ---- postproc for node group g ----
                            A = wp.tile([128, HID], f32, tag="A")
                            if ones_fast:
                                # table rows are pre-scaled by dinv[src]; out =
                                # dinv[d]*(agg + h_own) + b
                                tmp = wp.tile([128, HID], f32, tag="tmp")
                                nc.vector.tensor_tensor(out=tmp[:], in0=agg_ps[:],
                                                        in1=h_own[:, g, :],
                                                        op=mybir.AluOpType.add)
                                nc.vector.tensor_scalar(
                                    out=tmp[:], in0=tmp[:], scalar1=dinv_t[:, g:g + 1],
                                    scalar2=None, op0=mybir.AluOpType.mult)
                                nc.vector.tensor_tensor(out=A[:], in0=tmp[:], in1=brep_t[:, L, :],
                                                        op=mybir.AluOpType.add)
                            else:
                                nc.vector.tensor_tensor(out=A[:], in0=agg_ps[:], in1=brep_t[:, L, :],
                                                        op=mybir.AluOpType.add)
                                tmp = wp.tile([128, HID], f32, tag="tmp")
                                nc.vector.tensor_scalar(
                                    out=tmp[:], in0=h_own[:, g, :], scalar1=s2_t[:, g:g + 1],
                                    scalar2=None, op0=mybir.AluOpType.mult)
                                nc.vector.tensor_tensor(out=A[:], in0=A[:], in1=tmp[:],
                                                        op=mybir.AluOpType.add)
                            sq = wp.tile([128, HID], f32, tag="sq")
                            n2 = wp.tile([128, 1], f32, tag="n2")
                            nc.scalar.activation(out=sq[:], in_=A[:],
                                                 func=mybir.ActivationFunctionType.Square,
                                                 accum_out=n2[:])
                            nc.vector.tensor_scalar(out=n2[:], in0=n2[:], scalar1=EPS2,
                                                    scalar2=None, op0=mybir.AluOpType.max)
                            rt_ = wp.tile([128, 1], f32, tag="rt")
                            nc.scalar.activation(out=rt_[:], in_=n2[:],
                                                 func=mybir.ActivationFunctionType.Sqrt)
                            rinv = wp.tile([128, 1], f32, tag="rinv")
                            nc.vector.reciprocal(out=rinv[:], in_=rt_[:])
                            nc.scalar.activation(out=o_bufs[L][:, g, :], in_=A[:],
                                                 func=mybir.ActivationFunctionType.Relu,
                                                 scale=rinv[:])
                            if L < 2:
                                chain_write(L, g, o_bufs[L][:, g, :])

                # ---- head: y = [o1|o2|o3] @ W_lin + b ----
                y_v = t_y.rearrange("(g p k) -> g p k", p=128, k=NCLS)
                for g in range(NG):
                    y_ps = pp.tile([128, NCLS], f32, tag="hps")
                    for L in range(3):
                        ot_ps = pp2.tile([HID, 128], f32, tag="tps")
                        nc.tensor.transpose(out=ot_ps[:], in_=o_bufs[L][:, g, :], identity=eye_t[:])
                        ot_sb = wp.tile([HID, 128], f32, tag="otsb")
                        nc.vector.tensor_copy(out=ot_sb[:], in_=ot_ps[:])
                        nc.tensor.matmul(y_ps[:], lhsT=ot_sb[:], rhs=wl_ts[L][:],
                                         start=(L == 0), stop=(L == 2), skip_group_check=True)
                    y_sb = wp.tile([128, NCLS], f32, tag="ysb")
                    if debug_o1:
                        nc.vector.tensor_copy(out=y_sb[:], in_=o_bufs[dbgL][:, g, :NCLS])
                    else:
                        nc.vector.tensor_tensor(out=y_sb[:], in0=y_ps[:], in1=blrep_t[:],
                                                op=mybir.AluOpType.add)
                    nc.sync.dma_start(out=y_v[g], in_=y_sb[:])

    nc.compile()
    return nc


def _make_in_maps(meta, W1, b1, W2, b2, W3, b3, W_lin, b_lin):
    brep = np.stack([np.tile(np.asarray(b, np.float32)[None, :], (128, 1))
                     for b in (b1, b2, b3)])           # [3,128,HID]
    blrep = np.tile(np.asarray(b_lin, np.float32)[None, :], (128, 1))
    iota = np.tile(np.arange(W, dtype=np.float32)[None, :], (128, 1))
    eye = np.eye(128, dtype=np.float32)
    maps = []
    for c in range(NCORES):
        maps.append({
            "x_c": meta["x_pad"][c].reshape(-1),
            "idxb": meta["idx_blocks"][c],
            "cnorm": meta["cnorm_pt"][c],
            "dstrel": meta["dstrel_pt"][c],
            "s2": meta["s2_pt"][c],
            "dinv": meta["dinv_pt"][c],
            "w1": np.asarray(W1, np.float32), "w2": np.asarray(W2, np.float32),
            "w3": np.asarray(W3, np.float32), "wl": np.asarray(W_lin, np.float32),
            "brep": brep, "blrep": blrep, "iota": iota, "eye": eye,
        })
    return maps


def kernel(x, edge_index, edge_weights, W1, b1, W2, b2, W3, b3, W_lin, b_lin):
    meta = _host_prep(x, edge_index, edge_weights)
    key = ("prog", meta["ones_fast"])
    if key not in _cache:
        _cache[key] = _build_program(meta, ones_fast=meta["ones_fast"])
    nc = _cache[key]
    in_maps = _make_in_maps(meta, W1, b1, W2, b2, W3, b3, W_lin, b_lin)
    res = run_bass_kernel_spmd(nc, in_maps, core_ids=list(range(NCORES)))
    ys = [res.results[c]["y"].reshape(PC, NCLS)[:RC] for c in range(NCORES)]
    return np.concatenate(ys, axis=0).astype(np.float32)



# revision 15
# speedup vs baseline: 1.6343x; 1.6343x over previous
"""3-layer GCN node predictor on 8 Trainium2 NeuronCores (Bass/Tile SPMD).

Strategy (graph/data parallel, per sharding hint):
- Nodes sharded into 8 contiguous chunks (12544 padded rows per core); each
  core aggregates the in-edges of its own dst nodes.
- Per layer, the gather table T_L = o_{L-1} @ W_L ([100352, 64] fp32, 256B
  rows) is built shard-wise and AllGathered to every core's DRAM.
- Per-edge gather of T_L[src] uses gpsimd dma_gather (int16 indices ->
  4 table quarters of 25088 rows), round-robin over 4 SWDGE queues.
- Scatter-add uses TensorE: one-hot S [128 edges, 64 dst] built on DVE via
  is_equal against an iota row, matmul S.T @ msg accumulated in PSUM.
- Self loops are applied node-wise from the SBUF-resident own chunk.
"""
import numpy as np

import concourse.bass as bass
import concourse.bacc as bacc
import concourse.tile as tile
import concourse.mybir as mybir
from concourse.bass_utils import run_bass_kernel_spmd

NCORES = 8
N = 100000
E = 3200000
F_IN = 128
HID = 32
NCLS = 10
RC = 12500          # real nodes per core
PC = 12544          # padded nodes per core (98 * 128)
NP = PC * NCORES    # padded total nodes (100352)
Q4 = NP // 4        # table quarter rows (25088), int16-addressable
ELEM = 64           # table row elements (256B rows)
W = 64              # dst window
NWIN = PC // W      # 196 windows per core
SGW = 4             # windows per supergroup
NSG = NWIN // SGW   # 49
NG = PC // 128      # 98 node groups of 128
EPS2 = 1e-24

_cache = {}


def _host_prep(x, edge_index, edge_weights):
    src = np.asarray(edge_index[0], dtype=np.int64)
    dst = np.asarray(edge_index[1], dtype=np.int64)
    ew = np.asarray(edge_weights, dtype=np.float64)

    deg = np.bincount(dst, weights=ew, minlength=N) + 1.0
    dinv = np.where(deg > 0, 1.0 / np.sqrt(deg), 0.0)
    cnorm_e = (dinv[src] * ew * dinv[dst]).astype(np.float32)
    s2 = (dinv * dinv).astype(np.float32)

    psrc = (src // RC) * PC + (src % RC)          # padded global src ids

    per_core = []
    for c in range(NCORES):
        m = (dst >= RC * c) & (dst < RC * (c + 1))
        es = psrc[m]
        ed = dst[m] - RC * c
        en = cnorm_e[m]
        w_id = ed // W
        q_id = es // Q4
        order = np.lexsort((ed, q_id, w_id))      # sort by (w, q, dst)
        per_core.append((es[order], ed[order], en[order],
                         w_id[order], q_id[order]))

    # per (w, q) counts and max over cores
    counts = np.zeros((NCORES, NWIN, 4), dtype=np.int64)
    for c in range(NCORES):
        _, _, _, w_id, q_id = per_core[c]
        np.add.at(counts[c], (w_id, q_id), 1)
    cmax = counts.max(axis=0)
    t_wq = (cmax + 127) // 128                    # tiles per (w, q)
    for w_i in range(NWIN):
        if t_wq[w_i].sum() == 0:
            t_wq[w_i, 0] = 1

    # global tile order: (sg, q, w, k)
    tile_of = {}
    T_total = 0
    call_meta = []                                # (sg, q, t0, ntiles)
    for sg in range(NSG):
        for q in range(4):
            t0 = T_total
            for w_i in range(sg * SGW, (sg + 1) * SGW):
                for k in range(t_wq[w_i, q]):
                    tile_of[(w_i, q, k)] = T_total
                    T_total += 1
            call_meta.append((sg, q, t0, T_total - t0))

    # slot arrays
    idx16 = np.zeros((T_total * 128,), dtype=np.int16)
    cnorm = np.zeros((T_total * 128,), dtype=np.float32)
    dstrel = np.full((T_total * 128,), -1.0, dtype=np.float32)
    idx16_all = np.zeros((NCORES, T_total * 128), dtype=np.int16)
    cnorm_all = np.zeros((NCORES, T_total * 128), dtype=np.float32)
    dstrel_all = np.full((NCORES, T_total * 128), -1.0, dtype=np.float32)
    for c in range(NCORES):
        es, ed, en, w_id, q_id = per_core[c]
        # position within (w, q) run
        keys = w_id * 4 + q_id
        # edges already sorted by (w, q); rank within group:
        boundaries = np.flatnonzero(np.diff(keys, prepend=-1))
        ranks = np.arange(len(keys)) - np.repeat(boundaries, np.diff(np.append(boundaries, len(keys))))
        k_tile = ranks // 128
        k_part = ranks % 128
        gtile = np.array([tile_of[(w, q, k)] for (w, q, k) in zip(w_id, q_id, k_tile)])
        slot = gtile * 128 + k_part
        idx16_all[c, slot] = (es % Q4).astype(np.int16)
        cnorm_all[c, slot] = en
        dstrel_all[c, slot] = (ed - w_id * W).astype(np.float32)

    # device layouts
    # cnorm/dstrel resident [128, T]: flat p * T + t; slot = t*128 + p
    def to_pt(a):
        return np.ascontiguousarray(a.reshape(-1, T_total, 128).transpose(0, 2, 1)).reshape(NCORES, -1)

    cnorm_pt = to_pt(cnorm_all)
    dstrel_pt = to_pt(dstrel_all)

    # idx per supergroup: one [128, nts*8] int16 block per sg (p-major across
    # the whole sg) so the device loads ONE idx DMA per sg. Within the block,
    # each call's indices are wrapped [16, nidx/16] and replicated across the
    # 8 groups of 16 partitions, at column offset (t0-t0s)*8.
    idx_blocks = np.zeros((NCORES, T_total * 1024), dtype=np.int16)
    for sg in range(NSG):
        calls = [(s, q, t0, nt) for (s, q, t0, nt) in call_meta if s == sg]
        t0s = min(t0 for (_, _, t0, _) in calls)
        t1s = max(t0 + nt for (_, _, t0, nt) in calls)
        nts = t1s - t0s
        for c in range(NCORES):
            cols = np.zeros((128, nts * 8), dtype=np.int16)
            for (_, q, t0, nt) in calls:
                if nt == 0:
                    continue
                nidx = nt * 128
                blk = idx16_all[c, t0 * 128:(t0 + nt) * 128]
                wrp = blk.reshape(nidx // 16, 16).T          # [16, nidx/16]
                rep = np.tile(wrp, (8, 1))                   # [128, nidx/16]
                cols[:, (t0 - t0s) * 8:(t0 - t0s) * 8 + nt * 8] = rep
            idx_blocks[c, t0s * 1024:t1s * 1024] = cols.reshape(-1)

    # s2/dinv resident [128, NG]: flat p * NG + g ; node g*128+p
    def node_pt(v):
        pad = np.zeros((NCORES, PC), dtype=np.float32)
        for c in range(NCORES):
            pad[c, :RC] = v[RC * c:RC * (c + 1)]
        return np.ascontiguousarray(pad.reshape(NCORES, NG, 128).transpose(0, 2, 1)).reshape(NCORES, -1)

    s2_pt = node_pt(s2)
    dinv_pt = node_pt(dinv.astype(np.float32))
    ones_fast = bool(np.all(np.asarray(edge_weights) == 1.0))

    # x chunks
    x = np.asarray(x, dtype=np.float32)
    x_pad = np.zeros((NCORES, PC, F_IN), dtype=np.float32)
    for c in range(NCORES):
        x_pad[c, :RC] = x[RC * c:RC * (c + 1)]

    return dict(
        T_total=T_total, t_wq=t_wq, tile_of=tile_of, call_meta=call_meta,
        cnorm_pt=cnorm_pt, dstrel_pt=dstrel_pt, idx_blocks=idx_blocks,
        s2_pt=s2_pt, dinv_pt=dinv_pt, ones_fast=ones_fast, x_pad=x_pad,
    )


def _build_program(meta, reps=1, skip_gather=False, skip_scatter=False, ones_fast=False, debug_o1=False, dbgL=0, skip_allgather=False):
    T_total = meta["T_total"]
    t_wq = meta["t_wq"]
    tile_of = meta["tile_of"]
    call_meta = meta["call_meta"]
    f32 = mybir.dt.float32

    nc = bacc.Bacc("TRN2", target_bir_lowering=False, debug=False,
                   num_devices=NCORES, num_swdge_queues=4)

    t_x = nc.dram_tensor("x_c", [PC * F_IN], f32, kind="ExternalInput").ap()
    t_idx = nc.dram_tensor("idxb", [T_total * 1024], mybir.dt.int16, kind="ExternalInput").ap()
    t_cnorm = nc.dram_tensor("cnorm", [128 * T_total], f32, kind="ExternalInput").ap()
    t_dstrel = nc.dram_tensor("dstrel", [128 * T_total], f32, kind="ExternalInput").ap()
    t_s2 = nc.dram_tensor("s2", [128 * NG], f32, kind="ExternalInput").ap()
    t_dinv = nc.dram_tensor("dinv", [128 * NG], f32, kind="ExternalInput").ap()
    t_w1 = nc.dram_tensor("w1", [F_IN, HID], f32, kind="ExternalInput").ap()
    t_w2 = nc.dram_tensor("w2", [HID, HID], f32, kind="ExternalInput").ap()
    t_w3 = nc.dram_tensor("w3", [HID, HID], f32, kind="ExternalInput").ap()
    t_wl = nc.dram_tensor("wl", [3 * HID, NCLS], f32, kind="ExternalInput").ap()
    t_brep = nc.dram_tensor("brep", [3, 128, HID], f32, kind="ExternalInput").ap()
    t_blrep = nc.dram_tensor("blrep", [128, NCLS], f32, kind="ExternalInput").ap()
    t_iota = nc.dram_tensor("iota", [128, W], f32, kind="ExternalInput").ap()
    t_eye = nc.dram_tensor("eye", [128, 128], f32, kind="ExternalInput").ap()
    t_y = nc.dram_tensor("y", [PC * NCLS], f32, kind="ExternalOutput").ap()

    with tile.TileContext(nc) as tc:
        with (
            tc.tile_pool(name="const", bufs=1) as cp,
            tc.tile_pool(name="resident", bufs=1) as rp,
            tc.tile_pool(name="work", bufs=3) as wp,
            tc.tile_pool(name="idxp", bufs=3) as ip,
            tc.tile_pool(name="msgp", bufs=3) as mp,
            tc.tile_pool(name="sp", bufs=6) as spool,
            tc.tile_pool(name="psum", bufs=2, space="PSUM") as pp,
            tc.tile_pool(name="psum2", bufs=2, space="PSUM") as pp2,
            tc.tile_pool(name="dram", bufs=1, space="DRAM") as dp,
        ):
            # ---- constants / residents ----
            w1_t = cp.tile([F_IN, HID], f32); nc.sync.dma_start(out=w1_t[:], in_=t_w1[:])
            w2_t = cp.tile([HID, HID], f32); nc.sync.dma_start(out=w2_t[:], in_=t_w2[:])
            w3_t = cp.tile([HID, HID], f32); nc.sync.dma_start(out=w3_t[:], in_=t_w3[:])
            wl_ts = []
            for L in range(3):
                wt = cp.tile([HID, NCLS], f32, tag=f"wl{L}", name=f"wl{L}")
                nc.sync.dma_start(out=wt[:], in_=t_wl[HID * L:HID * (L + 1), :])
                wl_ts.append(wt)
            brep_t = cp.tile([128, 3, HID], f32)
            nc.sync.dma_start(out=brep_t[:], in_=t_brep.rearrange("l p h -> p l h"))
            blrep_t = cp.tile([128, NCLS], f32); nc.sync.dma_start(out=blrep_t[:], in_=t_blrep[:])
            iota_t = cp.tile([128, W], f32); nc.sync.dma_start(out=iota_t[:], in_=t_iota[:])
            eye_t = cp.tile([128, 128], f32); nc.sync.dma_start(out=eye_t[:], in_=t_eye[:])
            if not ones_fast:
                cnorm_t = rp.tile([128, T_total], f32)
                nc.sync.dma_start(out=cnorm_t[:], in_=t_cnorm.rearrange("(p t) -> p t", t=T_total))
                s2_t = rp.tile([128, NG], f32)
                nc.sync.dma_start(out=s2_t[:], in_=t_s2.rearrange("(p g) -> p g", g=NG))
            dstrel_t = rp.tile([128, T_total], f32)
            nc.sync.dma_start(out=dstrel_t[:], in_=t_dstrel.rearrange("(p t) -> p t", t=T_total))
            dinv_t = rp.tile([128, NG], f32)
            nc.sync.dma_start(out=dinv_t[:], in_=t_dinv.rearrange("(p g) -> p g", g=NG))

            h_own = rp.tile([128, NG, HID], f32)          # own chunk of current table (pre-pad)
            o_bufs = [rp.tile([128, NG, HID], f32, tag=f"o{L}", name=f"o{L}") for L in range(3)]

            in_bs = [dp.tile([PC * ELEM], f32, tag=f"inb{L}", name=f"inb{L}") for L in range(3)]

            if skip_allgather:
                # Pre-fill tables with finite data (x repeats) so gathers read
                # real values; outside the reps loop so slope timing excludes it.
                tables_static = [dp.tile([NP * ELEM], f32, tag=f"table{L}",
                                         name=f"table{L}") for L in range(3)]
                for L in range(3):
                    for q in range(4):
                        nc.sync.dma_start(
                            out=tables_static[L][PC * F_IN * q:PC * F_IN * (q + 1)],
                            in_=t_x[:])

            for _rep in range(reps):
                x_v = t_x.rearrange("(g p f) -> g p f", p=128, f=F_IN)
                y_v = t_y.rearrange("(g p k) -> g p k", p=128, k=NCLS)
                # Shared DRAM output tensors allow a single writer inst, so
                # each rep's AllGather needs its own table tensor.
                if skip_allgather:
                    tables = tables_static
                else:
                    tables = [dp.tile([NP * ELEM], f32, tag=f"table{L}_{_rep}",
                                      name=f"table{L}_{_rep}", addr_space="Shared")
                              for L in range(3)]

                def chain_write(L, g, o_ap):
                    """From o tile [128, HID] compute h = o @ W_{L+1}, write padded
                    row block to in_bs[L+1] and h_own."""
                    wn = [None, w2_t, w3_t][L + 1]
                    ot_ps = pp2.tile([HID, 128], f32, tag="tps")
                    nc.tensor.transpose(out=ot_ps[:], in_=o_ap, identity=eye_t[:])
                    ot_sb = wp.tile([HID, 128], f32, tag="otsb")
                    nc.vector.tensor_copy(out=ot_sb[:], in_=ot_ps[:])
                    h_ps = pp.tile([128, HID], f32, tag="hps")
                    nc.tensor.matmul(h_ps[:], lhsT=ot_sb[:], rhs=wn[:], start=True, stop=True)
                    h64 = wp.tile([128, ELEM], f32, tag="h64")
                    nc.vector.memset(h64[:, HID:], 0.0)
                    if ones_fast:
                        nc.vector.tensor_scalar(out=h64[:, :HID], in0=h_ps[:],
                                                scalar1=dinv_t[:, g:g + 1], scalar2=None,
                                                op0=mybir.AluOpType.mult)
                    else:
                        nc.vector.tensor_copy(out=h64[:, :HID], in_=h_ps[:])
                    nc.vector.tensor_copy(out=h_own[:, g, :], in_=h64[:, :HID])
                    nc.sync.dma_start(
                        out=in_bs[L + 1][:].rearrange("(g p e) -> g p e", p=128, e=ELEM)[g],
                        in_=h64[:])

                # ---- layer 1 table: h1 = x @ W1 ----
                for g in range(NG):
                    xt = wp.tile([128, F_IN], f32, tag="xt")
                    nc.sync.dma_start(out=xt[:], in_=x_v[g])
                    xT_ps = pp2.tile([128, 128], f32, tag="tps")
                    nc.tensor.transpose(out=xT_ps[:], in_=xt[:], identity=eye_t[:])
                    xT_sb = wp.tile([128, F_IN], f32, tag="xTsb")
                    nc.vector.tensor_copy(out=xT_sb[:], in_=xT_ps[:])
                    h_ps = pp.tile([128, HID], f32, tag="hps")
                    nc.tensor.matmul(h_ps[:], lhsT=xT_sb[:], rhs=w1_t[:], start=True, stop=True)
                    h64 = wp.tile([128, ELEM], f32, tag="h64")
                    nc.vector.memset(h64[:, HID:], 0.0)
                    if ones_fast:
                        nc.vector.tensor_scalar(out=h64[:, :HID], in0=h_ps[:],
                                                scalar1=dinv_t[:, g:g + 1], scalar2=None,
                                                op0=mybir.AluOpType.mult)
                    else:
                        nc.vector.tensor_copy(out=h64[:, :HID], in_=h_ps[:])
                    nc.vector.tensor_copy(out=h_own[:, g, :], in_=h64[:, :HID])
                    nc.sync.dma_start(
                        out=in_bs[0][:].rearrange("(g p e) -> g p e", p=128, e=ELEM)[g],
                        in_=h64[:])

                # ---- layers ----
                for L in range(3):
                    if not skip_allgather:
                        nc.gpsimd.collective_compute(
                            "AllGather", mybir.AluOpType.bypass,
                            replica_groups=[list(range(NCORES))],
                            ins=[in_bs[L][:]], outs=[tables[L][:]])
                    tab_q = [tables[L][:].rearrange("(n e) -> n e", e=ELEM)[Q4 * q:Q4 * (q + 1)]
                             for q in range(4)]

                    # supergroup tile extents
                    sg_t0 = [min(t0 for (s, q, t0, nt) in call_meta if s == sg)
                             for sg in range(NSG)]
                    sg_t1 = [max(t0 + nt for (s, q, t0, nt) in call_meta if s == sg)
                             for sg in range(NSG)]

                    for sg in range(NSG):
                        t0s, t1s = sg_t0[sg], sg_t1[sg]
                        nts = t1s - t0s
                        idxt = ip.tile([128, nts * 8], mybir.dt.int16, tag="idxt")
                        nc.sync.dma_start(
                            out=idxt[:],
                            in_=t_idx[t0s * 1024:t1s * 1024]
                                .rearrange("(p n) -> p n", p=128))
                        msg = mp.tile([128, nts, ELEM], f32, tag="msg")
                        for (s, q, t0, nt) in call_meta:
                            if s != sg or nt == 0:
                                continue
                            nidx = nt * 128
                            if not skip_gather:
                                nc.gpsimd.dma_gather(
                                    out_ap=msg[:, t0 - t0s:t0 - t0s + nt, :],
                                    in_ap=tab_q[q],
                                    idxs_ap=idxt[:, (t0 - t0s) * 8:(t0 - t0s + nt) * 8],
                                    num_idxs=nidx, num_idxs_reg=nidx,
                                    elem_size=ELEM, elem_step=ELEM,
                                    single_packet=False, queue_num=q)
                            if not ones_fast:
                                nc.vector.tensor_tensor(
                                    out=msg[:, t0 - t0s:t0 - t0s + nt, :HID],
                                    in0=msg[:, t0 - t0s:t0 - t0s + nt, :HID],
                                    in1=cnorm_t[:, t0:t0 + nt].unsqueeze(-1).to_broadcast([128, nt, HID]),
                                    op=mybir.AluOpType.mult)

                        # two psum groups (2 windows each) per supergroup
                        for half in range(SGW // 2):
                            agg_ps = pp.tile([128, HID], f32, tag="agg")
                            if skip_scatter:
                                nc.vector.memset(agg_ps[:], 0.0)
                            g = sg * 2 + half  # node group index (128 dst)
                            for wi in range(2):
                                w_i = sg * SGW + half * 2 + wi
                                tiles_w = []
                                for q in range(4):
                                    for k in range(t_wq[w_i, q]):
                                        tiles_w.append(tile_of[(w_i, q, k)])
                                # S tiles in runs of up to 8 consecutive
                                runs = []
                                for t in tiles_w:
                                    if runs and runs[-1][0] + runs[-1][1] == t and runs[-1][1] < 8:
                                        runs[-1] = (runs[-1][0], runs[-1][1] + 1)
                                    else:
                                        runs.append((t, 1))
                                s_tiles = {}
                                for (rt, rn) in (runs if not skip_scatter else []):
                                    st = spool.tile([128, 8, W], f32, tag="S")
                                    nc.vector.tensor_tensor(
                                        out=st[:, :rn, :],
                                        in0=dstrel_t[:, rt:rt + rn].unsqueeze(-1).to_broadcast([128, rn, W]),
                                        in1=iota_t[:].unsqueeze(1).to_broadcast([128, rn, W]),
                                        op=mybir.AluOpType.is_equal)
                                    for j in range(rn):
                                        s_tiles[rt + j] = st[:, j, :]
                                for ti, t in (enumerate(tiles_w) if not skip_scatter else []):
                                    nc.tensor.matmul(
                                        agg_ps[64 * wi:64 * (wi + 1), :],
                                        lhsT=s_tiles[t],
                                        rhs=msg[:, t - t0s, :HID],
                                        start=(ti == 0), stop=(ti == len(tiles_w) - 1),
                                        skip_group_check=True)
                            # ---- postproc for node group g ----
                            A = wp.tile([128, HID], f32, tag="A")
                            if ones_fast:
                                # table rows are pre-scaled by dinv[src]; out =
                                # dinv[d]*(agg + h_own) + b
                                tmp = wp.tile([128, HID], f32, tag="tmp")
                                nc.vector.tensor_tensor(out=tmp[:], in0=agg_ps[:],
                                                        in1=h_own[:, g, :],
                                                        op=mybir.AluOpType.add)
                                nc.vector.tensor_scalar(
                                    out=tmp[:], in0=tmp[:], scalar1=dinv_t[:, g:g + 1],
                                    scalar2=None, op0=mybir.AluOpType.mult)
                                nc.vector.tensor_tensor(out=A[:], in0=tmp[:], in1=brep_t[:, L, :],
                                                        op=mybir.AluOpType.add)
                            else:
                                nc.vector.tensor_tensor(out=A[:], in0=agg_ps[:], in1=brep_t[:, L, :],
                                                        op=mybir.AluOpType.add)
                                tmp = wp.tile([128, HID], f32, tag="tmp")
                                nc.vector.tensor_scalar(
                                    out=tmp[:], in0=h_own[:, g, :], scalar1=s2_t[:, g:g + 1],
                                    scalar2=None, op0=mybir.AluOpType.mult)
                                nc.vector.tensor_tensor(out=A[:], in0=A[:], in1=tmp[:],
                                                        op=mybir.AluOpType.add)
                            sq = wp.tile([128, HID], f32, tag="sq")
                            n2 = wp.tile([128, 1], f32, tag="n2")
                            nc.scalar.activation(out=sq[:], in_=A[:],
                                                 func=mybir.ActivationFunctionType.Square,
                                                 accum_out=n2[:])
                            nc.vector.tensor_scalar(out=n2[:], in0=n2[:], scalar1=EPS2,
                                                    scalar2=None, op0=mybir.AluOpType.max)
                            rt_ = wp.tile([128, 1], f32, tag="rt")
                            nc.scalar.activation(out=rt_[:], in_=n2[:],
                                                 func=mybir.ActivationFunctionType.Sqrt)
                            rinv = wp.tile([128, 1], f32, tag="rinv")
                            nc.vector.reciprocal(out=rinv[:], in_=rt_[:])
                            nc.scalar.activation(out=o_bufs[L][:, g, :], in_=A[:],
                                                 func=mybir.ActivationFunctionType.Relu,
                                                 scale=rinv[:])
                            if L < 2:
                                chain_write(L, g, o_bufs[L][:, g, :])
                            else:
                                # ---- head for group g: y = [o1|o2|o3] @ W_lin + b ----
                                y_ps = pp.tile([128, NCLS], f32, tag="hps")
                                for Lh in range(3):
                                    ot_ps = pp2.tile([HID, 128], f32, tag="tps")
                                    nc.tensor.transpose(out=ot_ps[:], in_=o_bufs[Lh][:, g, :], identity=eye_t[:])
                                    ot_sb = wp.tile([HID, 128], f32, tag="otsb")
                                    nc.vector.tensor_copy(out=ot_sb[:], in_=ot_ps[:])
                                    nc.tensor.matmul(y_ps[:], lhsT=ot_sb[:], rhs=wl_ts[Lh][:],
                                                     start=(Lh == 0), stop=(Lh == 2), skip_group_check=True)
                                y_sb = wp.tile([128, NCLS], f32, tag="ysb")
                                if debug_o1:
                                    nc.vector.tensor_copy(out=y_sb[:], in_=o_bufs[dbgL][:, g, :NCLS])
                                else:
                                    nc.vector.tensor_tensor(out=y_sb[:], in0=y_ps[:], in1=blrep_t[:],
                                                            op=mybir.AluOpType.add)
                                nc.sync.dma_start(out=y_v[g], in_=y_sb[:])

    nc.compile()
    return nc


def _make_in_maps(meta, W1, b1, W2, b2, W3, b3, W_lin, b_lin):
    brep = np.stack([np.tile(np.asarray(b, np.float32)[None, :], (128, 1))
                     for b in (b1, b2, b3)])           # [3,128,HID]
    blrep = np.tile(np.asarray(b_lin, np.float32)[None, :], (128, 1))
    iota = np.tile(np.arange(W, dtype=np.float32)[None, :], (128, 1))
    eye = np.eye(128, dtype=np.float32)
    maps = []
    for c in range(NCORES):
        maps.append({
            "x_c": meta["x_pad"][c].reshape(-1),
            "idxb": meta["idx_blocks"][c],
            "cnorm": meta["cnorm_pt"][c],
            "dstrel": meta["dstrel_pt"][c],
            "s2": meta["s2_pt"][c],
            "dinv": meta["dinv_pt"][c],
            "w1": np.asarray(W1, np.float32), "w2": np.asarray(W2, np.float32),
            "w3": np.asarray(W3, np.float32), "wl": np.asarray(W_lin, np.float32),
            "brep": brep, "blrep": blrep, "iota": iota, "eye": eye,
        })
    return maps


def kernel(x, edge_index, edge_weights, W1, b1, W2, b2, W3, b3, W_lin, b_lin):
    meta = _host_prep(x, edge_index, edge_weights)
    key = ("prog", meta["ones_fast"])
    if key not in _cache:
        _cache[key] = _build_program(meta, ones_fast=meta["ones_fast"])
    nc = _cache[key]
    in_maps = _make_in_maps(meta, W1, b1, W2, b2, W3, b3, W_lin, b_lin)
    res = run_bass_kernel_spmd(nc, in_maps, core_ids=list(range(NCORES)))
    ys = [res.results[c]["y"].reshape(PC, NCLS)[:RC] for c in range(NCORES)]
    return np.concatenate(ys, axis=0).astype(np.float32)



# revision 21
# speedup vs baseline: 2.0211x; 1.2366x over previous
"""3-layer GCN node predictor on 8 Trainium2 NeuronCores (Bass/Tile SPMD).

Strategy (graph/data parallel, per sharding hint):
- Nodes sharded into 8 contiguous chunks (12544 padded rows per core); each
  core aggregates the in-edges of its own dst nodes.
- Per layer, the gather table T_L = o_{L-1} @ W_L is built shard-wise in fp16
  ([100352, 128] rows of 256B, payload in the first 32 elements) and
  AllGathered into Shared DRAM on every core.
- Per-edge gather of T_L[src] uses gpsimd dma_gather (int16 indices into
  4 table quarters of 25088 rows), one SWDGE queue per quarter; edges are
  blocked by (dst-group of 128, quarter) and padded to 128-multiples at
  block granularity only.
- Scatter-add uses TensorE: one-hot S [128 edges, 128 dst] built on DVE in
  fp16 via is_equal against an iota row, matmul S.T @ msg accumulated in
  PSUM (fp32). Self loops are applied node-wise from SBUF (fp32).
- The classifier head runs per node-group inside layer 3's postproc so it
  hides under the layer-3 gathers.
"""
import numpy as np

import concourse.bass as bass
import concourse.bacc as bacc
import concourse.tile as tile
import concourse.mybir as mybir
from concourse.bass_utils import run_bass_kernel_spmd

NCORES = 8
N = 100000
E = 3200000
F_IN = 128
HID = 32
NCLS = 10
RC = 12500          # real nodes per core
PC = 12544          # padded nodes per core (98 * 128)
NP = PC * NCORES    # padded total nodes (100352)
Q4 = NP // 4        # table quarter rows (25088), int16-addressable
ELEMB = 128         # fp16 elements per table row (256B rows, payload [:32])
W = 128             # dst group width (= node group)
NG = PC // 128      # 98 node groups of 128
NPAIR = NG // 2     # 49 group pairs (one gather msg buffer per pair)
EPS2 = 1e-24

_cache = {}


def _host_prep(x, edge_index, edge_weights):
    src = np.asarray(edge_index[0], dtype=np.int64)
    dst = np.asarray(edge_index[1], dtype=np.int64)
    ew = np.asarray(edge_weights, dtype=np.float64)

    deg = np.bincount(dst, weights=ew, minlength=N) + 1.0
    dinv = np.where(deg > 0, 1.0 / np.sqrt(deg), 0.0)
    cnorm_e = (dinv[src] * ew * dinv[dst]).astype(np.float32)
    s2 = (dinv * dinv).astype(np.float32)

    psrc = (src // RC) * PC + (src % RC)          # padded global src ids

    per_core = []
    for c in range(NCORES):
        m = (dst >= RC * c) & (dst < RC * (c + 1))
        es = psrc[m]
        ed = dst[m] - RC * c
        en = cnorm_e[m]
        g_id = ed // W
        q_id = es // Q4
        order = np.lexsort((ed, q_id, g_id))      # sort by (g, q, dst)
        per_core.append((es[order], ed[order], en[order],
                         g_id[order], q_id[order]))

    # per (g, q) counts and max over cores
    counts = np.zeros((NCORES, NG, 4), dtype=np.int64)
    for c in range(NCORES):
        _, _, _, g_id, q_id = per_core[c]
        np.add.at(counts[c], (g_id, q_id), 1)
    cmax = counts.max(axis=0)
    t_gq = (cmax + 127) // 128                    # tiles per (g, q)
    for g in range(NG):
        if t_gq[g].sum() == 0:
            t_gq[g, 0] = 1

    # global tile order: (pair, q, g-within-pair, k) so each (pair, q) gather
    # call covers a contiguous tile range
    tile_of = {}
    T_total = 0
    call_meta = []                                # (pair, q, t0, ntiles)
    for pj in range(NPAIR):
        for q in range(4):
            t0 = T_total
            for g in (2 * pj, 2 * pj + 1):
                for k in range(t_gq[g, q]):
                    tile_of[(g, q, k)] = T_total
                    T_total += 1
            call_meta.append((pj, q, t0, T_total - t0))

    # slot arrays
    idx16_all = np.zeros((NCORES, T_total * 128), dtype=np.int16)
    cnorm_all = np.zeros((NCORES, T_total * 128), dtype=np.float32)
    dstrel_all = np.full((NCORES, T_total * 128), -1.0, dtype=np.float32)
    for c in range(NCORES):
        es, ed, en, g_id, q_id = per_core[c]
        # position within (g, q) run
        keys = g_id * 4 + q_id
        boundaries = np.flatnonzero(np.diff(keys, prepend=-1))
        ranks = np.arange(len(keys)) - np.repeat(boundaries, np.diff(np.append(boundaries, len(keys))))
        k_tile = ranks // 128
        k_part = ranks % 128
        gtile = np.array([tile_of[(g, q, k)] for (g, q, k) in zip(g_id, q_id, k_tile)])
        slot = gtile * 128 + k_part
        idx16_all[c, slot] = (es % Q4).astype(np.int16)
        cnorm_all[c, slot] = en
        dstrel_all[c, slot] = (ed - g_id * W).astype(np.float32)

    # device layouts
    # cnorm/dstrel resident [128, T]: flat p * T + t; slot = t*128 + p
    def to_pt(a, dt):
        return np.ascontiguousarray(
            a.reshape(-1, T_total, 128).transpose(0, 2, 1)).reshape(NCORES, -1).astype(dt)

    cnorm_pt = to_pt(cnorm_all, np.float16)
    dstrel_pt = to_pt(dstrel_all, np.float16)

    # idx per pair: one [128, nts*8] int16 block per pair (p-major across the
    # pair) so the device loads ONE idx DMA per pair. Within the block, each
    # call's indices are wrapped [16, nidx/16] and replicated across the 8
    # groups of 16 partitions, at column offset (t0-t0s)*8.
    idx_blocks = np.zeros((NCORES, T_total * 1024), dtype=np.int16)
    for pj in range(NPAIR):
        calls = [(p, q, t0, nt) for (p, q, t0, nt) in call_meta if p == pj]
        t0s = min(t0 for (_, _, t0, _) in calls)
        t1s = max(t0 + nt for (_, _, t0, nt) in calls)
        nts = t1s - t0s
        for c in range(NCORES):
            cols = np.zeros((128, nts * 8), dtype=np.int16)
            for (_, q, t0, nt) in calls:
                if nt == 0:
                    continue
                nidx = nt * 128
                blk = idx16_all[c, t0 * 128:(t0 + nt) * 128]
                wrp = blk.reshape(nidx // 16, 16).T          # [16, nidx/16]
                rep = np.tile(wrp, (8, 1))                   # [128, nidx/16]
                cols[:, (t0 - t0s) * 8:(t0 - t0s) * 8 + nt * 8] = rep
            idx_blocks[c, t0s * 1024:t1s * 1024] = cols.reshape(-1)

    # s2/dinv resident [128, NG]: flat p * NG + g ; node g*128+p
    def node_pt(v):
        pad = np.zeros((NCORES, PC), dtype=np.float32)
        for c in range(NCORES):
            pad[c, :RC] = v[RC * c:RC * (c + 1)]
        return np.ascontiguousarray(pad.reshape(NCORES, NG, 128).transpose(0, 2, 1)).reshape(NCORES, -1)

    s2_pt = node_pt(s2)
    dinv_pt = node_pt(dinv.astype(np.float32))
    ones_fast = bool(np.all(np.asarray(edge_weights) == 1.0))

    # x chunks
    x = np.asarray(x, dtype=np.float32)
    x_pad = np.zeros((NCORES, PC, F_IN), dtype=np.float32)
    for c in range(NCORES):
        x_pad[c, :RC] = x[RC * c:RC * (c + 1)]

    return dict(
        T_total=T_total, t_gq=t_gq, tile_of=tile_of, call_meta=call_meta,
        cnorm_pt=cnorm_pt, dstrel_pt=dstrel_pt, idx_blocks=idx_blocks,
        s2_pt=s2_pt, dinv_pt=dinv_pt, ones_fast=ones_fast, x_pad=x_pad,
    )


def _build_program(meta, reps=1, skip_gather=False, skip_scatter=False, ones_fast=False, debug_o1=False, dbgL=0, skip_allgather=False):
    T_total = meta["T_total"]
    t_gq = meta["t_gq"]
    TQMAX = int(t_gq.max())
    tile_of = meta["tile_of"]
    call_meta = meta["call_meta"]
    f32 = mybir.dt.float32
    f16 = mybir.dt.float16

    nc = bacc.Bacc("TRN2", target_bir_lowering=False, debug=False,
                   num_devices=NCORES, num_swdge_queues=4)

    t_x = nc.dram_tensor("x_c", [PC * F_IN], f32, kind="ExternalInput").ap()
    t_idx = nc.dram_tensor("idxb", [T_total * 1024], mybir.dt.int16, kind="ExternalInput").ap()
    t_cnorm = nc.dram_tensor("cnorm", [128 * T_total], f16, kind="ExternalInput").ap()
    t_dstrel = nc.dram_tensor("dstrel", [128 * T_total], f16, kind="ExternalInput").ap()
    t_s2 = nc.dram_tensor("s2", [128 * NG], f32, kind="ExternalInput").ap()
    t_dinv = nc.dram_tensor("dinv", [128 * NG], f32, kind="ExternalInput").ap()
    t_w1 = nc.dram_tensor("w1", [F_IN, HID], f32, kind="ExternalInput").ap()
    t_w2 = nc.dram_tensor("w2", [HID, HID], f32, kind="ExternalInput").ap()
    t_w3 = nc.dram_tensor("w3", [HID, HID], f32, kind="ExternalInput").ap()
    t_wl = nc.dram_tensor("wl", [3 * HID, NCLS], f32, kind="ExternalInput").ap()
    t_brep = nc.dram_tensor("brep", [3, 128, HID], f32, kind="ExternalInput").ap()
    t_blrep = nc.dram_tensor("blrep", [128, NCLS], f32, kind="ExternalInput").ap()
    t_iota = nc.dram_tensor("iota", [128, W], f16, kind="ExternalInput").ap()
    t_eye = nc.dram_tensor("eye", [128, 128], f32, kind="ExternalInput").ap()
    t_y = nc.dram_tensor("y", [PC * NCLS], f32, kind="ExternalOutput").ap()

    with tile.TileContext(nc) as tc:
        with (
            tc.tile_pool(name="const", bufs=1) as cp,
            tc.tile_pool(name="resident", bufs=1) as rp,
            tc.tile_pool(name="work", bufs=3) as wp,
            tc.tile_pool(name="idxp", bufs=3) as ip,
            tc.tile_pool(name="msgp", bufs=3) as mp,
            tc.tile_pool(name="sp", bufs=6) as spool,
            tc.tile_pool(name="psum", bufs=2, space="PSUM") as pp,
            tc.tile_pool(name="psum2", bufs=2, space="PSUM") as pp2,
            tc.tile_pool(name="dram", bufs=1, space="DRAM") as dp,
        ):
            # ---- constants / residents ----
            w1_t = cp.tile([F_IN, HID], f32); nc.sync.dma_start(out=w1_t[:], in_=t_w1[:])
            w2_t = cp.tile([HID, HID], f32); nc.sync.dma_start(out=w2_t[:], in_=t_w2[:])
            w3_t = cp.tile([HID, HID], f32); nc.sync.dma_start(out=w3_t[:], in_=t_w3[:])
            wl_ts = []
            for L in range(3):
                wt = cp.tile([HID, NCLS], f32, tag=f"wl{L}", name=f"wl{L}")
                nc.sync.dma_start(out=wt[:], in_=t_wl[HID * L:HID * (L + 1), :])
                wl_ts.append(wt)
            brep_t = cp.tile([128, 3, HID], f32)
            nc.sync.dma_start(out=brep_t[:], in_=t_brep.rearrange("l p h -> p l h"))
            blrep_t = cp.tile([128, NCLS], f32); nc.sync.dma_start(out=blrep_t[:], in_=t_blrep[:])
            iota_t = cp.tile([128, W], f16); nc.sync.dma_start(out=iota_t[:], in_=t_iota[:])
            eye_t = cp.tile([128, 128], f32); nc.sync.dma_start(out=eye_t[:], in_=t_eye[:])
            if not ones_fast:
                cnorm_t = rp.tile([128, T_total], f16)
                nc.sync.dma_start(out=cnorm_t[:], in_=t_cnorm.rearrange("(p t) -> p t", t=T_total))
                s2_t = rp.tile([128, NG], f32)
                nc.sync.dma_start(out=s2_t[:], in_=t_s2.rearrange("(p g) -> p g", g=NG))
            dstrel_t = rp.tile([128, T_total], f16)
            nc.sync.dma_start(out=dstrel_t[:], in_=t_dstrel.rearrange("(p t) -> p t", t=T_total))
            dinv_t = rp.tile([128, NG], f32)
            nc.sync.dma_start(out=dinv_t[:], in_=t_dinv.rearrange("(p g) -> p g", g=NG))

            h_own = rp.tile([128, NG, HID], f32)          # own chunk (fp32, scaled)
            o_bufs = [rp.tile([128, NG, HID], f32, tag=f"o{L}", name=f"o{L}") for L in range(3)]

            in_bs = [dp.tile([PC * ELEMB], f16, tag=f"inb{L}", name=f"inb{L}") for L in range(3)]

            if skip_allgather:
                # Pre-fill tables with finite data so gathers read real
                # values; outside the reps loop so slope timing excludes it.
                tables_static = [dp.tile([NP * ELEMB], f16, tag=f"table{L}",
                                         name=f"table{L}") for L in range(3)]
                xf = wp.tile([128, ELEMB], f16, tag="xf16")
                nc.vector.memset(xf[:], 0.5)
                for L in range(3):
                    v = tables_static[L][:].rearrange("(g p e) -> g p e", p=128, e=ELEMB)
                    for g in range(NG * NCORES):
                        nc.sync.dma_start(out=v[g], in_=xf[:])

            for _rep in range(reps):
                x_v = t_x.rearrange("(g p f) -> g p f", p=128, f=F_IN)
                y_v = t_y.rearrange("(g p k) -> g p k", p=128, k=NCLS)
                # Shared DRAM output tensors allow a single writer inst, so
                # each rep's AllGather needs its own table tensor.
                if skip_allgather:
                    tables = tables_static
                else:
                    tables = [dp.tile([NP * ELEMB], f16, tag=f"table{L}_{_rep}",
                                      name=f"table{L}_{_rep}", addr_space="Shared")
                              for L in range(3)]

                def emit_h(L, g, h_ps):
                    """From fp32 psum h [128, HID]: scale by dinv (ones_fast),
                    keep fp32 copy in h_own, write fp16 row block to in_bs[L]."""
                    if ones_fast:
                        nc.vector.tensor_scalar(out=h_own[:, g, :], in0=h_ps[:],
                                                scalar1=dinv_t[:, g:g + 1], scalar2=None,
                                                op0=mybir.AluOpType.mult)
                    else:
                        nc.vector.tensor_copy(out=h_own[:, g, :], in_=h_ps[:])
                    h128 = wp.tile([128, ELEMB], f16, tag="h128")
                    nc.vector.tensor_copy(out=h128[:, :HID], in_=h_own[:, g, :])
                    # ACT-issued HWDGE: keeps the SP FIFO free for idx loads
                    # (HWDGE DMAs execute in FIFO order per issuing engine).
                    nc.scalar.dma_start(
                        out=in_bs[L][:].rearrange("(g p e) -> g p e", p=128, e=ELEMB)[g],
                        in_=h128[:])

                def chain_write(L, g, o_ap):
                    """From o tile [128, HID] compute h = o @ W_{L+1}, emit."""
                    wn = [None, w2_t, w3_t][L + 1]
                    ot_ps = pp2.tile([HID, 128], f32, tag="tps")
                    nc.tensor.transpose(out=ot_ps[:], in_=o_ap, identity=eye_t[:])
                    ot_sb = wp.tile([HID, 128], f32, tag="otsb")
                    nc.vector.tensor_copy(out=ot_sb[:], in_=ot_ps[:])
                    h_ps = pp.tile([128, HID], f32, tag="hps")
                    nc.tensor.matmul(h_ps[:], lhsT=ot_sb[:], rhs=wn[:], start=True, stop=True)
                    emit_h(L + 1, g, h_ps)

                # ---- layer 1 table: h1 = x @ W1 ----
                for g in range(NG):
                    xt = wp.tile([128, F_IN], f32, tag="xt")
                    nc.sync.dma_start(out=xt[:], in_=x_v[g])
                    xT_ps = pp2.tile([128, 128], f32, tag="tps")
                    nc.tensor.transpose(out=xT_ps[:], in_=xt[:], identity=eye_t[:])
                    xT_sb = wp.tile([128, F_IN], f32, tag="xTsb")
                    nc.vector.tensor_copy(out=xT_sb[:], in_=xT_ps[:])
                    h_ps = pp.tile([128, HID], f32, tag="hps")
                    nc.tensor.matmul(h_ps[:], lhsT=xT_sb[:], rhs=w1_t[:], start=True, stop=True)
                    emit_h(0, g, h_ps)

                # ---- layers ----
                for L in range(3):
                    if not skip_allgather:
                        nc.gpsimd.collective_compute(
                            "AllGather", mybir.AluOpType.bypass,
                            replica_groups=[list(range(NCORES))],
                            ins=[in_bs[L][:]], outs=[tables[L][:]])
                    tab_q = [tables[L][:].rearrange("(n e) -> n e", e=ELEMB)[Q4 * q:Q4 * (q + 1)]
                             for q in range(4)]

                    # pair tile extents
                    pr_t0 = [min(t0 for (p, q, t0, nt) in call_meta if p == pj)
                             for pj in range(NPAIR)]
                    pr_t1 = [max(t0 + nt for (p, q, t0, nt) in call_meta if p == pj)
                             for pj in range(NPAIR)]

                    for pj in range(NPAIR):
                        t0s, t1s = pr_t0[pj], pr_t1[pj]
                        nts = t1s - t0s
                        idxt = ip.tile([128, nts * 8], mybir.dt.int16, tag="idxt")
                        nc.sync.dma_start(
                            out=idxt[:],
                            in_=t_idx[t0s * 1024:t1s * 1024]
                                .rearrange("(p n) -> p n", p=128))
                        msg = mp.tile([128, nts, ELEMB], f16, tag="msg")
                        for (p, q, t0, nt) in call_meta:
                            if p != pj or nt == 0:
                                continue
                            nidx = nt * 128
                            if not skip_gather:
                                nc.gpsimd.dma_gather(
                                    out_ap=msg[:, t0 - t0s:t0 - t0s + nt, :],
                                    in_ap=tab_q[q],
                                    idxs_ap=idxt[:, (t0 - t0s) * 8:(t0 - t0s + nt) * 8],
                                    num_idxs=nidx, num_idxs_reg=nidx,
                                    elem_size=ELEMB, elem_step=ELEMB,
                                    single_packet=False, queue_num=q)
                            if not ones_fast:
                                nc.vector.tensor_tensor(
                                    out=msg[:, t0 - t0s:t0 - t0s + nt, :HID],
                                    in0=msg[:, t0 - t0s:t0 - t0s + nt, :HID],
                                    in1=cnorm_t[:, t0:t0 + nt].unsqueeze(-1).to_broadcast([128, nt, HID]),
                                    op=mybir.AluOpType.mult)

                        for g in (2 * pj, 2 * pj + 1):
                            agg_ps = pp.tile([128, HID], f32, tag="agg")
                            if skip_scatter:
                                nc.vector.memset(agg_ps[:], 0.0)
                            # per (g, q): tiles are contiguous; one is_equal
                            # builds the one-hot S run, then the matmuls.
                            tiles_g = []       # (global tile, S ap)
                            for q in range(4):
                                rn = t_gq[g, q]
                                if rn == 0 or skip_scatter:
                                    continue
                                rt = tile_of[(g, q, 0)]
                                st = spool.tile([128, TQMAX, W], f16, tag="S")
                                nc.vector.tensor_tensor(
                                    out=st[:, :rn, :],
                                    in0=dstrel_t[:, rt:rt + rn].unsqueeze(-1).to_broadcast([128, rn, W]),
                                    in1=iota_t[:].unsqueeze(1).to_broadcast([128, rn, W]),
                                    op=mybir.AluOpType.is_equal)
                                for j in range(rn):
                                    tiles_g.append((rt + j, st[:, j, :]))
                            for ti, (t, s_ap) in enumerate(tiles_g):
                                nc.tensor.matmul(
                                    agg_ps[:],
                                    lhsT=s_ap,
                                    rhs=msg[:, t - t0s, :HID],
                                    start=(ti == 0), stop=(ti == len(tiles_g) - 1),
                                    skip_group_check=True)
                            # ---- postproc for node group g ----
                            A = wp.tile([128, HID], f32, tag="A")
                            if ones_fast:
                                # table rows pre-scaled by dinv[src]; out =
                                # dinv[d]*(agg + h_own) + b
                                tmp = wp.tile([128, HID], f32, tag="tmp")
                                nc.vector.tensor_tensor(out=tmp[:], in0=agg_ps[:],
                                                        in1=h_own[:, g, :],
                                                        op=mybir.AluOpType.add)
                                nc.vector.tensor_scalar(
                                    out=tmp[:], in0=tmp[:], scalar1=dinv_t[:, g:g + 1],
                                    scalar2=None, op0=mybir.AluOpType.mult)
                                nc.vector.tensor_tensor(out=A[:], in0=tmp[:], in1=brep_t[:, L, :],
                                                        op=mybir.AluOpType.add)
                            else:
                                nc.vector.tensor_tensor(out=A[:], in0=agg_ps[:], in1=brep_t[:, L, :],
                                                        op=mybir.AluOpType.add)
                                tmp = wp.tile([128, HID], f32, tag="tmp")
                                nc.vector.tensor_scalar(
                                    out=tmp[:], in0=h_own[:, g, :], scalar1=s2_t[:, g:g + 1],
                                    scalar2=None, op0=mybir.AluOpType.mult)
                                nc.vector.tensor_tensor(out=A[:], in0=A[:], in1=tmp[:],
                                                        op=mybir.AluOpType.add)
                            sq = wp.tile([128, HID], f32, tag="sq")
                            n2 = wp.tile([128, 1], f32, tag="n2")
                            nc.scalar.activation(out=sq[:], in_=A[:],
                                                 func=mybir.ActivationFunctionType.Square,
                                                 accum_out=n2[:])
                            nc.vector.tensor_scalar(out=n2[:], in0=n2[:], scalar1=EPS2,
                                                    scalar2=None, op0=mybir.AluOpType.max)
                            rt_ = wp.tile([128, 1], f32, tag="rt")
                            nc.scalar.activation(out=rt_[:], in_=n2[:],
                                                 func=mybir.ActivationFunctionType.Sqrt)
                            rinv = wp.tile([128, 1], f32, tag="rinv")
                            nc.vector.reciprocal(out=rinv[:], in_=rt_[:])
                            nc.scalar.activation(out=o_bufs[L][:, g, :], in_=A[:],
                                                 func=mybir.ActivationFunctionType.Relu,
                                                 scale=rinv[:])
                            if L < 2:
                                chain_write(L, g, o_bufs[L][:, g, :])
                            else:
                                # ---- head for group g: y = [o1|o2|o3] @ W_lin + b ----
                                y_ps = pp.tile([128, NCLS], f32, tag="hps")
                                for Lh in range(3):
                                    ot_ps = pp2.tile([HID, 128], f32, tag="tps")
                                    nc.tensor.transpose(out=ot_ps[:], in_=o_bufs[Lh][:, g, :], identity=eye_t[:])
                                    ot_sb = wp.tile([HID, 128], f32, tag="otsb")
                                    nc.vector.tensor_copy(out=ot_sb[:], in_=ot_ps[:])
                                    nc.tensor.matmul(y_ps[:], lhsT=ot_sb[:], rhs=wl_ts[Lh][:],
                                                     start=(Lh == 0), stop=(Lh == 2), skip_group_check=True)
                                y_sb = wp.tile([128, NCLS], f32, tag="ysb")
                                if debug_o1:
                                    nc.vector.tensor_copy(out=y_sb[:], in_=o_bufs[dbgL][:, g, :NCLS])
                                else:
                                    nc.vector.tensor_tensor(out=y_sb[:], in0=y_ps[:], in1=blrep_t[:],
                                                            op=mybir.AluOpType.add)
                                nc.scalar.dma_start(out=y_v[g], in_=y_sb[:])

    nc.compile()
    return nc


def _make_in_maps(meta, W1, b1, W2, b2, W3, b3, W_lin, b_lin):
    brep = np.stack([np.tile(np.asarray(b, np.float32)[None, :], (128, 1))
                     for b in (b1, b2, b3)])           # [3,128,HID]
    blrep = np.tile(np.asarray(b_lin, np.float32)[None, :], (128, 1))
    iota = np.tile(np.arange(W, dtype=np.float16)[None, :], (128, 1))
    eye = np.eye(128, dtype=np.float32)
    maps = []
    for c in range(NCORES):
        maps.append({
            "x_c": meta["x_pad"][c].reshape(-1),
            "idxb": meta["idx_blocks"][c],
            "cnorm": meta["cnorm_pt"][c],
            "dstrel": meta["dstrel_pt"][c],
            "s2": meta["s2_pt"][c],
            "dinv": meta["dinv_pt"][c],
            "w1": np.asarray(W1, np.float32), "w2": np.asarray(W2, np.float32),
            "w3": np.asarray(W3, np.float32), "wl": np.asarray(W_lin, np.float32),
            "brep": brep, "blrep": blrep, "iota": iota, "eye": eye,
        })
    return maps


def kernel(x, edge_index, edge_weights, W1, b1, W2, b2, W3, b3, W_lin, b_lin):
    meta = _host_prep(x, edge_index, edge_weights)
    key = ("prog", meta["ones_fast"])
    if key not in _cache:
        _cache[key] = _build_program(meta, ones_fast=meta["ones_fast"])
    nc = _cache[key]
    in_maps = _make_in_maps(meta, W1, b1, W2, b2, W3, b3, W_lin, b_lin)
    res = run_bass_kernel_spmd(nc, in_maps, core_ids=list(range(NCORES)))
    ys = [res.results[c]["y"].reshape(PC, NCLS)[:RC] for c in range(NCORES)]
    return np.concatenate(ys, axis=0).astype(np.float32)
